# revision 25
# baseline (speedup 1.0000x reference)
"""Causal multi-head attention (RMSNorm + QKV + causal softmax + out-proj)
for Trainium2, sharded over 8 NeuronCores.

Sharding: data-parallel over batch (2) x tensor-parallel over head-groups
(16 heads -> 4 groups of 4). Core c = 4*b + hg computes
    partial_out[b] = Attn_heads[4hg:4hg+4](x[b]) @ Wo[256hg:256hg+256, :]
and the host sums the 4 head-group partials per batch (the TP unshard).

v2: bf16 end-to-end on device (f32 PSUM accumulation), halving HBM
traffic and removing all PE transposes of activations:
  - host pre-transposes x -> xt [dim, seq] bf16 and also ships x [seq, dim]
    bf16 (stats only); gamma is folded into Wq/Wk/Wv rows on the host.
  - RMSNorm stats via ACT square+accum on x tiles; s = 32/||x|| per token;
    s broadcast along partitions via PE outer-product (ones (x) s_row),
    xnT = xt * sbcast on DVE (bf16 2x mode).
  - Q^T/K^T (d-major) and V (seq-major) projections from xnT, bf16 matmuls.
  - attention per head pair: S^T = K^T.T Q^T (two K=64 matmuls packed via
    tile_position), P^T = exp(S^T/8 + maskbias) ACT psum->sbuf bf16,
    causal via block skipping + triangular bf16 mask multiply on diagonal
    windows, O^T = Vaug.T P^T with a ones-column giving row sums,
    normalize by DVE reciprocal (read from PSUM) + PE broadcast + multiply.
  - out = A @ Wo accumulated over the two 128-row halves of Wo, written
    to HBM as bf16; host upcasts and sums the 4 head-group partials.
"""
import os
import sys

for _p in ("/opt/trn_rl_repo", os.path.expanduser("~/.axon_site/_ro/trn_rl_repo")):
    if os.path.isdir(_p) and _p not in sys.path:
        sys.path.insert(0, _p)

import numpy as np

B = 2
N = 2048
DIM = 1024
HEADS = 16
DH = 64
SCALE = DH ** -0.5   # 0.125
NCORES = 8
NGROUPS = 4          # head groups (tensor parallel)
HPC = HEADS // NGROUPS  # 4 heads per core
P = 128
RC = 4               # row chunks of 512 for projections / q-chunks
QCHUNK = 512
NKB = N // P         # 16 key blocks
REPS = 1             # timing aid: emit the compute body REPS times


def _build():
    import concourse.bass as bass
    import concourse.mybir as mybir
    import concourse.tile as tile
    from concourse import bacc

    dt = mybir.dt
    f32 = dt.float32
    f32r = dt.float32r
    bf16 = dt.bfloat16
    AF = mybir.ActivationFunctionType
    ALU = mybir.AluOpType

    nc = bacc.Bacc("TRN2", target_bir_lowering=False, debug=False,
                   num_devices=NCORES)

    x_d = nc.dram_tensor("x", [N, DIM], bf16, kind="ExternalInput")
    xt_d = nc.dram_tensor("xt", [DIM, N], bf16, kind="ExternalInput")
    wq_d = nc.dram_tensor("wq", [DIM, HPC * DH], bf16, kind="ExternalInput")
    wk_d = nc.dram_tensor("wk", [DIM, HPC * DH], bf16, kind="ExternalInput")
    wv_d = nc.dram_tensor("wv", [DIM, HPC * DH], bf16, kind="ExternalInput")
    wo_d = nc.dram_tensor("wo", [HPC * DH, DIM], bf16, kind="ExternalInput")
    mb_d = nc.dram_tensor("maskbias", [P, NKB], f32, kind="ExternalInput")
    tri_d = nc.dram_tensor("tri", [P, P], bf16, kind="ExternalInput")
    id_d = nc.dram_tensor("ident", [P, P], f32, kind="ExternalInput")
    on_d = nc.dram_tensor("onesin", [1, DH], f32, kind="ExternalInput")
    sel_d = nc.dram_tensor("sel", [4, 4 * P], bf16, kind="ExternalInput")
    vo_d = nc.dram_tensor("vones", [P, NKB * HPC], bf16, kind="ExternalInput")
    out_d = nc.dram_tensor("out", [N, DIM], bf16, kind="ExternalOutput")

    with tile.TileContext(nc) as tc:
        with (
            tc.tile_pool(name="consts", bufs=1) as consts,
            tc.tile_pool(name="wpool", bufs=1) as wpool,
            tc.tile_pool(name="big", bufs=1) as big,
        ):
            # ---- constant / weight loads
            ident = consts.tile([P, P], f32)
            nc.gpsimd.dma_start(ident[:], id_d[:])
            tri = consts.tile([P, P], bf16)
            nc.gpsimd.dma_start(tri[:], tri_d[:])
            maskb = consts.tile([P, NKB], f32)
            nc.sync.dma_start(maskb[:], mb_d[:])
            onesr = consts.tile([1, DH], f32r)
            nc.gpsimd.dma_start(onesr[:], on_d[:])
            sel = consts.tile([4, 4, P], bf16)
            nc.gpsimd.dma_start(
                sel[:], sel_d.ap().rearrange("p (j c) -> p j c", c=P))

            wq = wpool.tile([P, 8, HPC * DH], bf16)
            wk = wpool.tile([P, 8, HPC * DH], bf16)
            wv = wpool.tile([P, 8, HPC * DH], bf16)
            wo = wpool.tile([P, 2, DIM], bf16)

            # ---- persistent activations
            qt = big.tile([P, 2, N], bf16)     # Q^T: [d-of-pair, hp, seq]
            kt = big.tile([P, 2, N], bf16)
            vt = big.tile([P, NKB, HPC, DH + 1], bf16)   # V rows + ones col
            nc.gpsimd.dma_start(
                vt[:, :, :, DH:DH + 1],
                vo_d.ap().rearrange("p (kb h) -> p kb h", h=HPC).unsqueeze(3))
            a0 = big.tile([P, N], bf16)        # A^T for head pair 0
            a1 = big.tile([P, N], bf16)
            ss = big.tile([P, 16], f32)        # row sum-of-squares
            sfac = big.tile([P, 16], f32)      # 32 / max(sqrt(ss), 1e-12)

            for _rep in range(REPS):
                # ===== merged pipeline: per row-chunk rc, do norm (stats +
                # PE-broadcast scale) + projections from xt, then attention
                # for q-chunk qc=rc, then out-projection for qc. PSUM
                # budget: ps1(2x1) + S(2x2) + O(1x2) = 8 banks.
                with (
                    tc.tile_pool(name="xin", bufs=4) as xin,
                    tc.tile_pool(name="sq", bufs=1) as sqp,
                    tc.tile_pool(name="xtp", bufs=3) as xtp,
                    tc.tile_pool(name="xnp", bufs=2) as xnp,
                    tc.tile_pool(name="sbp", bufs=2) as sbp,
                    tc.tile_pool(name="pt", bufs=4) as ptp,
                    tc.tile_pool(name="nrm", bufs=2) as nrm,
                    tc.tile_pool(name="outp", bufs=2) as outp,
                    tc.tile_pool(name="ps1", bufs=2, space="PSUM") as ps1,
                    tc.tile_pool(name="sps", bufs=2, space="PSUM") as sps,
                    tc.tile_pool(name="ops", bufs=1, space="PSUM") as ops,
                ):
                    def emit_outproj(qc_done, half=None):
                        rows = range(4 * qc_done, 4 * qc_done + 4)
                        if half is not None:
                            rows = rows[:2] if half == 0 else rows[2:]
                        for r in rows:
                            rs = slice(r * P, (r + 1) * P)
                            orow = outp.tile([P, DIM], bf16, tag="orow")
                            for cc in range(2):
                                ps = ps1.tile([P, QCHUNK], f32, tag="ps1")
                                for hp2, a in ((0, a0), (1, a1)):
                                    nc.tensor.matmul(
                                        ps[:], a[:, rs],
                                        wo[:, hp2, cc * QCHUNK:(cc + 1) * QCHUNK],
                                        start=(hp2 == 0), stop=(hp2 == 1))
                                ocol = slice(cc * QCHUNK, (cc + 1) * QCHUNK)
                                if cc == 0:
                                    nc.scalar.activation(orow[:, ocol], ps[:],
                                                         AF.Copy)
                                else:
                                    nc.vector.tensor_copy(orow[:, ocol], ps[:])
                            eng = nc.sync if r % 2 == 0 else nc.gpsimd
                            eng.dma_start(out_d[rs, :], orow[:])

                    def emit_xtc(rc):
                        ns = slice(rc * QCHUNK, (rc + 1) * QCHUNK)
                        xtc = xtp.tile([P, 8, QCHUNK], bf16, tag="xt")
                        nc.gpsimd.dma_start(
                            xtc[:],
                            xt_d.ap().rearrange("(c p) n -> p c n", p=P)[:, :, ns])
                        return xtc

                    def emit_stats_a(rc):
                        # squares -> sqrt -> 1/ -> *32 (ACT + DVE only)
                        for t in range(4):
                            ti = rc * 4 + t
                            xrow = xin.tile([P, DIM], bf16, tag="x")
                            nc.sync.dma_start(xrow[:], x_d[ti * P:(ti + 1) * P, :])
                            scr = sqp.tile([P, DIM], bf16, tag="sq")
                            nc.scalar.activation(scr[:], xrow[:], AF.Square,
                                                 accum_out=ss[:, ti:ti + 1])
                        sl = slice(rc * 4, rc * 4 + 4)
                        nc.scalar.activation(sfac[:, sl], ss[:, sl], AF.Sqrt)
                        nc.vector.tensor_scalar(out=sfac[:, sl], in0=sfac[:, sl],
                                                scalar1=1e-12, scalar2=None,
                                                op0=ALU.max)
                        nc.vector.reciprocal(sfac[:, sl], sfac[:, sl])
                        nc.vector.tensor_scalar(out=sfac[:, sl], in0=sfac[:, sl],
                                                scalar1=float(DIM ** 0.5),
                                                scalar2=None, op0=ALU.mult)

                    def emit_stats_b(rc):
                        # transpose sfac cols to a row (chunk j on partition
                        # j) then PE selector broadcast -> sbc_sb [128, 512]
                        sl = slice(rc * 4, rc * 4 + 4)
                        sT = ps1.tile([4, P], f32, tag="ps1")
                        nc.tensor.transpose(sT[:], sfac[:, sl], ident[:])
                        strow = sbp.tile([4, P], bf16, tag="strow")
                        nc.vector.tensor_copy(strow[:], sT[:])
                        sbc = ps1.tile([P, QCHUNK], f32, tag="ps1")
                        for j in range(4):
                            nc.tensor.matmul(sbc[:, j * P:(j + 1) * P],
                                             sel[:, j, :], strow[:],
                                             start=True, stop=True)
                        sbc_sb = sbp.tile([P, QCHUNK], bf16, tag="sbc")
                        nc.scalar.activation(sbc_sb[:], sbc[:], AF.Copy)
                        return sbc_sb

                    def emit_proj(rc):
                        # Q/K/V projections on RAW x^T; RMSNorm scales are
                        # applied during PSUM evacuation so the PE never
                        # waits on the stats chain.
                        xtc = xtcs.pop(rc)
                        sbc_sb = emit_stats_b(rc)
                        ns = slice(rc * QCHUNK, (rc + 1) * QCHUNK)
                        for w, dst in ((wq, qt), (wk, kt)):
                            for cc in range(2):
                                ps = ps1.tile([P, QCHUNK], f32, tag="ps1")
                                for k in range(8):
                                    nc.tensor.matmul(
                                        ps[:], w[:, k, cc * P:(cc + 1) * P],
                                        xtc[:, k, :],
                                        start=(k == 0), stop=(k == 7))
                                nc.vector.tensor_tensor(dst[:, cc, ns], ps[:],
                                                        sbc_sb[:], ALU.mult)
                        for t in range(4):
                            kb = rc * 4 + t
                            ps = ps1.tile([P, HPC * DH], f32, tag="ps1")
                            for k in range(8):
                                nc.tensor.matmul(
                                    ps[:], xtc[:, k, t * P:(t + 1) * P],
                                    wv[:, k, :],
                                    start=(k == 0), stop=(k == 7))
                            nc.vector.tensor_scalar(
                                out=vt[:, kb, :, 0:DH],
                                in0=ps[:].rearrange("p (h d) -> p h d", d=DH),
                                scalar1=sfac[:, kb:kb + 1], scalar2=None,
                                op0=ALU.mult)

                    def emit_attn(rc, fill_pe=None):
                        # S/exp/PV software-pipelined: PV(kb-1) is emitted
                        # after S(kb) so the PE never stalls on the exp of
                        # the current kb. fill_pe() is emitted before the
                        # final bt broadcast to hide the reciprocal latency.
                        qc = rc
                        qs = slice(qc * QCHUNK, (qc + 1) * QCHUNK)
                        nkb = 4 * qc + 4
                        for hp, adst in ((0, a0), (1, a1)):
                            ot = ops.tile([DH + 1, 2, QCHUNK], f32, tag="o")
                            pts = {}
                            for kb in range(nkb):
                                ks = slice(kb * P, (kb + 1) * P)
                                o = max(0, kb * P - qc * QCHUNK)
                                qso = slice(qc * QCHUNK + o, (qc + 1) * QCHUNK)
                                st = sps.tile([P, 2, QCHUNK], f32, tag="s")
                                for h in range(2):
                                    nc.tensor.matmul(
                                        st[:, h, o:],
                                        kt[h * DH:(h + 1) * DH, hp, ks],
                                        qt[h * DH:(h + 1) * DH, hp, qso],
                                        start=True, stop=True,
                                        tile_position=(h * DH, 0))
                                pt = ptp.tile([P, 2, QCHUNK], bf16, tag="pt")
                                nc.scalar.activation(pt[:, :, o:], st[:, :, o:],
                                                     AF.Exp, scale=SCALE,
                                                     bias=maskb[:, kb:kb + 1])
                                if kb >= 4 * qc:  # diagonal block: tri mask
                                    nc.vector.tensor_tensor(
                                        pt[:, :, o:o + P], pt[:, :, o:o + P],
                                        tri[:, None, :].broadcast_to([P, 2, P]),
                                        ALU.mult)
                                pts[kb] = (pt, o)
                                if kb > 0:
                                    ptp_, op_ = pts.pop(kb - 1)
                                    for h in range(2):
                                        nc.tensor.matmul(
                                            ot[:, h, op_:],
                                            vt[:, kb - 1, 2 * hp + h, :],
                                            ptp_[:, h, op_:],
                                            start=(kb - 1 == 0), stop=False,
                                            skip_group_check=True)
                            ptl, ol = pts.pop(nkb - 1)
                            for h in range(2):
                                nc.tensor.matmul(
                                    ot[:, h, ol:], vt[:, nkb - 1, 2 * hp + h, :],
                                    ptl[:, h, ol:],
                                    start=(nkb == 1), stop=True,
                                    skip_group_check=True)
                            # normalize: A = O[0:64] * (1 / O[64])
                            rec = nrm.tile([1, 2, QCHUNK], f32r, tag="rec")
                            with nc.allow_low_precision(reason="f32r softmax recip"):
                                nc.vector.reciprocal(rec[:], ot[DH:DH + 1, :, :])
                            if fill_pe is not None:
                                fill_pe(hp)
                            bt = sps.tile([DH, 2, QCHUNK], f32, tag="s")
                            for h in range(2):
                                nc.tensor.matmul(bt[:, h, :], onesr[:],
                                                 rec[0:1, h, :],
                                                 start=True, stop=True)
                            btsb = nrm.tile([DH, 2, QCHUNK], bf16, tag="btsb")
                            nc.vector.tensor_copy(btsb[:], bt[:])
                            nc.vector.tensor_tensor(adst[0:DH, qs],
                                                    ot[0:DH, 0, :],
                                                    btsb[:, 0, :], ALU.mult)
                            ashq = nrm.tile([DH, QCHUNK], bf16, tag="ashq")
                            nc.vector.tensor_tensor(ashq[:], ot[0:DH, 1, :],
                                                    btsb[:, 1, :], ALU.mult)
                            nc.sync.dma_start(adst[DH:2 * DH, qs], ashq[:])

                    # ---- pipelined emission: xt prefetch 2 ahead, stats one
                    # chunk ahead, out-projection deferred one chunk and
                    # split across the two head-pair tails to fill the
                    # reciprocal latency before each bt broadcast.
                    xtcs = {0: emit_xtc(0)}
                    emit_stats_a(0)
                    if _rep == 0:
                        nc.sync.dma_start(
                            wq[:], wq_d.ap().rearrange("(k p) c -> p k c", p=P))
                        nc.sync.dma_start(
                            wk[:], wk_d.ap().rearrange("(k p) c -> p k c", p=P))
                        nc.sync.dma_start(
                            wv[:], wv_d.ap().rearrange("(k p) c -> p k c", p=P))
                        nc.sync.dma_start(
                            wo[:], wo_d.ap().rearrange("(hp p) c -> p hp c", p=P))
                    xtcs[1] = emit_xtc(1)
                    for rc in range(RC):
                        emit_proj(rc)
                        if rc + 2 < RC:
                            xtcs[rc + 2] = emit_xtc(rc + 2)
                        if rc + 1 < RC:
                            emit_stats_a(rc + 1)
                        fill = ((lambda hp, r=rc: emit_outproj(r - 1, half=hp))
                                if rc > 0 else None)
                        emit_attn(rc, fill_pe=fill)
                    emit_outproj(RC - 1)

    nc.compile()
    return nc


_CACHE = {}


def _get_nc():
    if "nc" not in _CACHE:
        _CACHE["nc"] = _build()
    return _CACHE["nc"]


def kernel(x, mask, gamma, Wq, Wkv, Wo):
    import ml_dtypes
    from concourse import bass_utils

    bf16 = ml_dtypes.bfloat16

    x = np.asarray(x, dtype=np.float32)
    mask = np.asarray(mask)
    gamma = np.asarray(gamma, dtype=np.float32)
    Wq = np.asarray(Wq, dtype=np.float32) * gamma[:, None]
    Wkv = np.asarray(Wkv, dtype=np.float32) * gamma[:, None]
    Wo = np.asarray(Wo, dtype=np.float32)

    tri = (np.arange(P)[None, :] >= np.arange(P)[:, None]).astype(bf16)
    ident = np.eye(P, dtype=np.float32)

    in_maps = []
    for c in range(NCORES):
        b, hg = divmod(c, NGROUPS)
        cs = slice(hg * HPC * DH, (hg + 1) * HPC * DH)
        mb = np.where(mask[b], 0.0, -1e30).astype(np.float32)
        xb = x[b].astype(bf16)
        in_maps.append({
            "x": xb,
            "xt": np.ascontiguousarray(xb.T),
            "wq": np.ascontiguousarray(Wq[:, cs]).astype(bf16),
            "wk": np.ascontiguousarray(Wkv[:, :DIM][:, cs]).astype(bf16),
            "wv": np.ascontiguousarray(Wkv[:, DIM:][:, cs]).astype(bf16),
            "wo": np.ascontiguousarray(Wo[cs, :]).astype(bf16),
            "maskbias": np.ascontiguousarray(mb.reshape(NKB, P).T),
            "tri": tri,
            "ident": ident,
            "onesin": np.ones((1, DH), dtype=np.float32),
            "sel": np.kron(np.eye(4, dtype=np.float32),
                           np.ones((1, P), dtype=np.float32)).astype(bf16),
            "vones": np.ones((P, NKB * HPC), dtype=bf16),
        })

    nc = _get_nc()
    _CACHE["last_in_maps"] = in_maps
    res = bass_utils.run_bass_kernel_spmd(nc, in_maps, core_ids=list(range(NCORES)))
    out = np.zeros((B, N, DIM), dtype=np.float32)
    for c in range(NCORES):
        b = c // NGROUPS
        out[b] += res.results[c]["out"].astype(np.float32)
    return out


# revision 26
# speedup vs baseline: 1.4964x; 1.4964x over previous
"""Causal multi-head attention (RMSNorm + QKV + causal softmax + out-proj)
for Trainium2, sharded over 8 NeuronCores.

Sharding: data-parallel over batch (2) x tensor-parallel over head-groups
(16 heads -> 4 groups of 4). Core c = 4*b + hg computes
    partial_out[b] = Attn_heads[4hg:4hg+4](x[b]) @ Wo[256hg:256hg+256, :]
and the host sums the 4 head-group partials per batch (the TP unshard).

v2: bf16 end-to-end on device (f32 PSUM accumulation), halving HBM
traffic and removing all PE transposes of activations:
  - host pre-transposes x -> xt [dim, seq] bf16 and also ships x [seq, dim]
    bf16 (stats only); gamma is folded into Wq/Wk/Wv rows on the host.
  - RMSNorm stats via ACT square+accum on x tiles; s = 32/||x|| per token;
    s broadcast along partitions via PE outer-product (ones (x) s_row),
    xnT = xt * sbcast on DVE (bf16 2x mode).
  - Q^T/K^T (d-major) and V (seq-major) projections from xnT, bf16 matmuls.
  - attention per head pair: S^T = K^T.T Q^T (two K=64 matmuls packed via
    tile_position), P^T = exp(S^T/8 + maskbias) ACT psum->sbuf bf16,
    causal via block skipping + triangular bf16 mask multiply on diagonal
    windows, O^T = Vaug.T P^T with a ones-column giving row sums,
    normalize by DVE reciprocal (read from PSUM) + PE broadcast + multiply.
  - out = A @ Wo accumulated over the two 128-row halves of Wo, written
    to HBM as bf16; host upcasts and sums the 4 head-group partials.
"""
import os
import sys

for _p in ("/opt/trn_rl_repo", os.path.expanduser("~/.axon_site/_ro/trn_rl_repo")):
    if os.path.isdir(_p) and _p not in sys.path:
        sys.path.insert(0, _p)

import numpy as np

B = 2
N = 2048
DIM = 1024
HEADS = 16
DH = 64
SCALE = DH ** -0.5   # 0.125
NCORES = 8
NGROUPS = 4          # head groups (tensor parallel)
HPC = HEADS // NGROUPS  # 4 heads per core
P = 128
RC = 4               # row chunks of 512 for projections / q-chunks
QCHUNK = 512
NKB = N // P         # 16 key blocks
REPS = 1             # timing aid: emit the compute body REPS times


def _build():
    import concourse.bass as bass
    import concourse.mybir as mybir
    import concourse.tile as tile
    from concourse import bacc

    dt = mybir.dt
    f32 = dt.float32
    f32r = dt.float32r
    bf16 = dt.bfloat16
    AF = mybir.ActivationFunctionType
    ALU = mybir.AluOpType

    nc = bacc.Bacc("TRN2", target_bir_lowering=False, debug=False,
                   num_devices=NCORES)

    x_d = nc.dram_tensor("x", [N, DIM], bf16, kind="ExternalInput")
    xt_d = nc.dram_tensor("xt", [DIM, N], bf16, kind="ExternalInput")
    wq_d = nc.dram_tensor("wq", [DIM, HPC * DH], bf16, kind="ExternalInput")
    wk_d = nc.dram_tensor("wk", [DIM, HPC * DH], bf16, kind="ExternalInput")
    wv_d = nc.dram_tensor("wv", [DIM, HPC * DH], bf16, kind="ExternalInput")
    wo_d = nc.dram_tensor("wo", [HPC * DH, DIM], bf16, kind="ExternalInput")
    mb_d = nc.dram_tensor("maskbias", [P, NKB], f32, kind="ExternalInput")
    tri_d = nc.dram_tensor("tri", [P, P], bf16, kind="ExternalInput")
    id_d = nc.dram_tensor("ident", [P, P], f32, kind="ExternalInput")
    on_d = nc.dram_tensor("onesin", [1, DH], f32, kind="ExternalInput")
    sel_d = nc.dram_tensor("sel", [4, 4 * P], bf16, kind="ExternalInput")
    vo_d = nc.dram_tensor("vones", [P, NKB * HPC], bf16, kind="ExternalInput")
    out_d = nc.dram_tensor("out", [N, DIM], bf16, kind="ExternalOutput")

    with tile.TileContext(nc) as tc:
        with (
            tc.tile_pool(name="consts", bufs=1) as consts,
            tc.tile_pool(name="wpool", bufs=1) as wpool,
            tc.tile_pool(name="big", bufs=1) as big,
        ):
            # ---- constant / weight loads
            ident = consts.tile([P, P], f32)
            nc.gpsimd.dma_start(ident[:], id_d[:])
            tri = consts.tile([P, P], bf16)
            nc.gpsimd.dma_start(tri[:], tri_d[:])
            maskb = consts.tile([P, NKB], f32)
            nc.sync.dma_start(maskb[:], mb_d[:])
            onesr = consts.tile([1, DH], f32r)
            nc.gpsimd.dma_start(onesr[:], on_d[:])
            sel = consts.tile([4, 4, P], bf16)
            nc.gpsimd.dma_start(
                sel[:], sel_d.ap().rearrange("p (j c) -> p j c", c=P))

            wq = wpool.tile([P, 8, HPC * DH], bf16)
            wk = wpool.tile([P, 8, HPC * DH], bf16)
            wv = wpool.tile([P, 8, HPC * DH], bf16)
            wo = wpool.tile([P, 2, DIM], bf16)

            # ---- persistent activations
            qt = big.tile([P, 2, N], bf16)     # Q^T: [d-of-pair, hp, seq]
            kt = big.tile([P, 2, N], bf16)
            vt = big.tile([P, NKB, HPC, DH + 1], bf16)   # V rows + ones col
            nc.gpsimd.dma_start(
                vt[:, :, :, DH:DH + 1],
                vo_d.ap().rearrange("p (kb h) -> p kb h", h=HPC).unsqueeze(3))
            a0 = big.tile([P, N], bf16)        # A^T for head pair 0
            a1 = big.tile([P, N], bf16)
            ss = big.tile([P, 16], f32)        # row sum-of-squares
            sfac = big.tile([P, 16], f32)      # 32 / max(sqrt(ss), 1e-12)

            # ===== merged pipeline: per row-chunk rc, do norm (stats +
            # PE-broadcast scale) + projections from xt, then attention
            # for q-chunk qc=rc, then out-projection for qc. PSUM
            # budget: ps1(2x1) + S(2x2) + O(1x2) = 8 banks. Pools live
            # OUTSIDE the rep loop so multi-rep timing NEFFs pipeline
            # across bodies like a production steady state.
            with (
                tc.tile_pool(name="xin", bufs=4) as xin,
                tc.tile_pool(name="sq", bufs=1) as sqp,
                tc.tile_pool(name="xtp", bufs=3) as xtp,
                tc.tile_pool(name="xnp", bufs=2) as xnp,
                tc.tile_pool(name="sbp", bufs=2) as sbp,
                tc.tile_pool(name="pt", bufs=4) as ptp,
                tc.tile_pool(name="nrm", bufs=2) as nrm,
                tc.tile_pool(name="outp", bufs=2) as outp,
                tc.tile_pool(name="ps1", bufs=2, space="PSUM") as ps1,
                tc.tile_pool(name="sps", bufs=2, space="PSUM") as sps,
                tc.tile_pool(name="ops", bufs=1, space="PSUM") as ops,
            ):
                for _rep in range(REPS):
                    def emit_outproj(qc_done, half=None):
                        rows = range(4 * qc_done, 4 * qc_done + 4)
                        if half is not None:
                            rows = rows[:2] if half == 0 else rows[2:]
                        for r in rows:
                            rs = slice(r * P, (r + 1) * P)
                            orow = outp.tile([P, DIM], bf16, tag="orow")
                            for cc in range(2):
                                ps = ps1.tile([P, QCHUNK], f32, tag="ps1")
                                for hp2, a in ((0, a0), (1, a1)):
                                    nc.tensor.matmul(
                                        ps[:], a[:, rs],
                                        wo[:, hp2, cc * QCHUNK:(cc + 1) * QCHUNK],
                                        start=(hp2 == 0), stop=(hp2 == 1))
                                ocol = slice(cc * QCHUNK, (cc + 1) * QCHUNK)
                                if cc == 0:
                                    nc.scalar.activation(orow[:, ocol], ps[:],
                                                         AF.Copy)
                                else:
                                    nc.vector.tensor_copy(orow[:, ocol], ps[:])
                            eng = nc.sync if r % 2 == 0 else nc.gpsimd
                            eng.dma_start(out_d[rs, :], orow[:])

                    def emit_xtc(rc):
                        ns = slice(rc * QCHUNK, (rc + 1) * QCHUNK)
                        xtc = xtp.tile([P, 8, QCHUNK], bf16, tag="xt")
                        nc.gpsimd.dma_start(
                            xtc[:],
                            xt_d.ap().rearrange("(c p) n -> p c n", p=P)[:, :, ns])
                        return xtc

                    def emit_stats_a(rc):
                        # squares -> sqrt -> 1/ -> *32 (ACT + DVE only)
                        for t in range(4):
                            ti = rc * 4 + t
                            xrow = xin.tile([P, DIM], bf16, tag="x")
                            nc.sync.dma_start(xrow[:], x_d[ti * P:(ti + 1) * P, :])
                            scr = sqp.tile([P, DIM], bf16, tag="sq")
                            nc.scalar.activation(scr[:], xrow[:], AF.Square,
                                                 accum_out=ss[:, ti:ti + 1])
                        sl = slice(rc * 4, rc * 4 + 4)
                        nc.scalar.activation(sfac[:, sl], ss[:, sl], AF.Sqrt)
                        nc.vector.tensor_scalar(out=sfac[:, sl], in0=sfac[:, sl],
                                                scalar1=1e-12, scalar2=None,
                                                op0=ALU.max)
                        nc.vector.reciprocal(sfac[:, sl], sfac[:, sl])
                        nc.vector.tensor_scalar(out=sfac[:, sl], in0=sfac[:, sl],
                                                scalar1=float(DIM ** 0.5),
                                                scalar2=None, op0=ALU.mult)

                    def emit_stats_b(rc):
                        # transpose sfac cols to a row (chunk j on partition
                        # j) then PE selector broadcast -> sbc_sb [128, 512]
                        sl = slice(rc * 4, rc * 4 + 4)
                        sT = ps1.tile([4, P], f32, tag="ps1")
                        nc.tensor.transpose(sT[:], sfac[:, sl], ident[:])
                        strow = sbp.tile([4, P], bf16, tag="strow")
                        nc.vector.tensor_copy(strow[:], sT[:])
                        sbc = ps1.tile([P, QCHUNK], f32, tag="ps1")
                        for j in range(4):
                            nc.tensor.matmul(sbc[:, j * P:(j + 1) * P],
                                             sel[:, j, :], strow[:],
                                             start=True, stop=True)
                        sbc_sb = sbp.tile([P, QCHUNK], bf16, tag="sbc")
                        nc.scalar.activation(sbc_sb[:], sbc[:], AF.Copy)
                        return sbc_sb

                    def emit_proj(rc):
                        # Q/K/V projections on RAW x^T; RMSNorm scales are
                        # applied during PSUM evacuation so the PE never
                        # waits on the stats chain.
                        xtc = xtcs.pop(rc)
                        sbc_sb = emit_stats_b(rc)
                        ns = slice(rc * QCHUNK, (rc + 1) * QCHUNK)
                        for w, dst in ((wq, qt), (wk, kt)):
                            for cc in range(2):
                                ps = ps1.tile([P, QCHUNK], f32, tag="ps1")
                                for k in range(8):
                                    nc.tensor.matmul(
                                        ps[:], w[:, k, cc * P:(cc + 1) * P],
                                        xtc[:, k, :],
                                        start=(k == 0), stop=(k == 7))
                                nc.vector.tensor_tensor(dst[:, cc, ns], ps[:],
                                                        sbc_sb[:], ALU.mult)
                        for t in range(4):
                            kb = rc * 4 + t
                            ps = ps1.tile([P, HPC * DH], f32, tag="ps1")
                            for k in range(8):
                                nc.tensor.matmul(
                                    ps[:], xtc[:, k, t * P:(t + 1) * P],
                                    wv[:, k, :],
                                    start=(k == 0), stop=(k == 7))
                            nc.vector.tensor_scalar(
                                out=vt[:, kb, :, 0:DH],
                                in0=ps[:].rearrange("p (h d) -> p h d", d=DH),
                                scalar1=sfac[:, kb:kb + 1], scalar2=None,
                                op0=ALU.mult)

                    def emit_attn(rc, fill_pe=None):
                        # S/exp/PV software-pipelined: PV(kb-1) is emitted
                        # after S(kb) so the PE never stalls on the exp of
                        # the current kb. fill_pe() is emitted before the
                        # final bt broadcast to hide the reciprocal latency.
                        qc = rc
                        qs = slice(qc * QCHUNK, (qc + 1) * QCHUNK)
                        nkb = 4 * qc + 4
                        for hp, adst in ((0, a0), (1, a1)):
                            ot = ops.tile([DH + 1, 2, QCHUNK], f32, tag="o")
                            pts = {}
                            for kb in range(nkb):
                                ks = slice(kb * P, (kb + 1) * P)
                                o = max(0, kb * P - qc * QCHUNK)
                                qso = slice(qc * QCHUNK + o, (qc + 1) * QCHUNK)
                                st = sps.tile([P, 2, QCHUNK], f32, tag="s")
                                for h in range(2):
                                    nc.tensor.matmul(
                                        st[:, h, o:],
                                        kt[h * DH:(h + 1) * DH, hp, ks],
                                        qt[h * DH:(h + 1) * DH, hp, qso],
                                        start=True, stop=True,
                                        tile_position=(h * DH, 0))
                                pt = ptp.tile([P, 2, QCHUNK], bf16, tag="pt")
                                nc.scalar.activation(pt[:, :, o:], st[:, :, o:],
                                                     AF.Exp, scale=SCALE,
                                                     bias=maskb[:, kb:kb + 1])
                                if kb >= 4 * qc:  # diagonal block: tri mask
                                    nc.vector.tensor_tensor(
                                        pt[:, :, o:o + P], pt[:, :, o:o + P],
                                        tri[:, None, :].broadcast_to([P, 2, P]),
                                        ALU.mult)
                                pts[kb] = (pt, o)
                                if kb > 0:
                                    ptp_, op_ = pts.pop(kb - 1)
                                    for h in range(2):
                                        nc.tensor.matmul(
                                            ot[:, h, op_:],
                                            vt[:, kb - 1, 2 * hp + h, :],
                                            ptp_[:, h, op_:],
                                            start=(kb - 1 == 0), stop=False,
                                            skip_group_check=True)
                            ptl, ol = pts.pop(nkb - 1)
                            for h in range(2):
                                nc.tensor.matmul(
                                    ot[:, h, ol:], vt[:, nkb - 1, 2 * hp + h, :],
                                    ptl[:, h, ol:],
                                    start=(nkb == 1), stop=True,
                                    skip_group_check=True)
                            # normalize: A = O[0:64] * (1 / O[64])
                            rec = nrm.tile([1, 2, QCHUNK], f32r, tag="rec")
                            with nc.allow_low_precision(reason="f32r softmax recip"):
                                nc.vector.reciprocal(rec[:], ot[DH:DH + 1, :, :])
                            if fill_pe is not None:
                                fill_pe(hp)
                            bt = sps.tile([DH, 2, QCHUNK], f32, tag="s")
                            for h in range(2):
                                nc.tensor.matmul(bt[:, h, :], onesr[:],
                                                 rec[0:1, h, :],
                                                 start=True, stop=True)
                            btsb = nrm.tile([DH, 2, QCHUNK], bf16, tag="btsb")
                            nc.vector.tensor_copy(btsb[:], bt[:])
                            nc.vector.tensor_tensor(adst[0:DH, qs],
                                                    ot[0:DH, 0, :],
                                                    btsb[:, 0, :], ALU.mult)
                            ashq = nrm.tile([DH, QCHUNK], bf16, tag="ashq")
                            nc.vector.tensor_tensor(ashq[:], ot[0:DH, 1, :],
                                                    btsb[:, 1, :], ALU.mult)
                            nc.sync.dma_start(adst[DH:2 * DH, qs], ashq[:])

                    # ---- pipelined emission: xt prefetch 2 ahead, stats one
                    # chunk ahead, out-projection deferred one chunk and
                    # split across the two head-pair tails to fill the
                    # reciprocal latency before each bt broadcast.
                    xtcs = {0: emit_xtc(0)}
                    emit_stats_a(0)
                    if _rep == 0:
                        nc.sync.dma_start(
                            wq[:], wq_d.ap().rearrange("(k p) c -> p k c", p=P))
                        nc.sync.dma_start(
                            wk[:], wk_d.ap().rearrange("(k p) c -> p k c", p=P))
                        nc.sync.dma_start(
                            wv[:], wv_d.ap().rearrange("(k p) c -> p k c", p=P))
                        nc.sync.dma_start(
                            wo[:], wo_d.ap().rearrange("(hp p) c -> p hp c", p=P))
                    xtcs[1] = emit_xtc(1)
                    for rc in range(RC):
                        emit_proj(rc)
                        if rc + 2 < RC:
                            xtcs[rc + 2] = emit_xtc(rc + 2)
                        if rc + 1 < RC:
                            emit_stats_a(rc + 1)
                        fill = ((lambda hp, r=rc: emit_outproj(r - 1, half=hp))
                                if rc > 0 else None)
                        emit_attn(rc, fill_pe=fill)
                    emit_outproj(RC - 1)

    nc.compile()
    return nc


_CACHE = {}


def _get_nc():
    if "nc" not in _CACHE:
        _CACHE["nc"] = _build()
    return _CACHE["nc"]


def kernel(x, mask, gamma, Wq, Wkv, Wo):
    import ml_dtypes
    from concourse import bass_utils

    bf16 = ml_dtypes.bfloat16

    x = np.asarray(x, dtype=np.float32)
    mask = np.asarray(mask)
    gamma = np.asarray(gamma, dtype=np.float32)
    Wq = np.asarray(Wq, dtype=np.float32) * gamma[:, None]
    Wkv = np.asarray(Wkv, dtype=np.float32) * gamma[:, None]
    Wo = np.asarray(Wo, dtype=np.float32)

    tri = (np.arange(P)[None, :] >= np.arange(P)[:, None]).astype(bf16)
    ident = np.eye(P, dtype=np.float32)

    in_maps = []
    for c in range(NCORES):
        b, hg = divmod(c, NGROUPS)
        cs = slice(hg * HPC * DH, (hg + 1) * HPC * DH)
        mb = np.where(mask[b], 0.0, -1e30).astype(np.float32)
        xb = x[b].astype(bf16)
        in_maps.append({
            "x": xb,
            "xt": np.ascontiguousarray(xb.T),
            "wq": np.ascontiguousarray(Wq[:, cs]).astype(bf16),
            "wk": np.ascontiguousarray(Wkv[:, :DIM][:, cs]).astype(bf16),
            "wv": np.ascontiguousarray(Wkv[:, DIM:][:, cs]).astype(bf16),
            "wo": np.ascontiguousarray(Wo[cs, :]).astype(bf16),
            "maskbias": np.ascontiguousarray(mb.reshape(NKB, P).T),
            "tri": tri,
            "ident": ident,
            "onesin": np.ones((1, DH), dtype=np.float32),
            "sel": np.kron(np.eye(4, dtype=np.float32),
                           np.ones((1, P), dtype=np.float32)).astype(bf16),
            "vones": np.ones((P, NKB * HPC), dtype=bf16),
        })

    nc = _get_nc()
    _CACHE["last_in_maps"] = in_maps
    res = bass_utils.run_bass_kernel_spmd(nc, in_maps, core_ids=list(range(NCORES)))
    out = np.zeros((B, N, DIM), dtype=np.float32)
    for c in range(NCORES):
        b = c // NGROUPS
        out[b] += res.results[c]["out"].astype(np.float32)
    return out
